# revision 22
# baseline (speedup 1.0000x reference)
"""MoE (8 experts, top-2, SwiGLU FFN) Trainium2 kernel.

Sharding: data-parallel over tokens on 4 of the 8 visible cores. Each
core gets T/4 = 1024 tokens, processed as two 512-token halves (PSUM
tiles and fp32 matmul moving-operand limits are sized for 512), sharing
each expert's weight tiles across both halves. Per half: router (fp32
matmul + softmax + top-2 via max/second-max thresholding) and all 8
experts' FFNs (bf16 matmuls with fp32 PSUM accumulation), accumulating
cw-weighted expert outputs on-chip. Host only reshapes/transposes inputs
and concatenates the 4 output slices.

Why 4 cores: per-execute dispatch cost on the axon PJRT path has a
~1.0ms floor that is flat for 1-4 devices but ~250us higher at 8
(measured with a 4KB kernel), while the per-core kernel body (~400us at
1024 tokens) stays fully hidden under that floor in pipelined execution.
Fewer cores -> less per-execute protocol work, same wall-clock.

I/O packing: dispatch cost also scales with the number of I/O buffers
(~22us/buffer), so inputs travel as 2 tensors:
  xs [128, DT*TLOC + 1152] f32  per-core token slice x^T (router needs
                                true fp32), then rwt | b1 | b3 | b2
  wb [E, 128, 3, 4096] bf16     per-expert w1|w3|w2, SBUF-ready layout
FFN weights are bf16: fp32 PSUM accumulation keeps end-to-end error at
~5e-3 (gate is 2e-2) and it halves both host->device and HBM->SBUF bytes.

Schedule notes (cost-model driven):
 - A few discarded bf16 matmuls warm the PE (HAM ramp) before the fp32
   router so the router runs at full clock.
 - DMA issue order: sm, x (per-d-tile chunks), then per-expert w1, w3, w2
   - so the first matmuls of each stage start as soon as their first
   operand lands.
 - The router->combine-weight chain (transpose, softmax, top-2) runs
   entirely on DVE/ACT (32x32 stream transposes + tiny partition-shift
   DMAs on the gpsimd queue), so the PE stream never interleaves with it.
 - Output is written per (half, t-tile, d-chunk) to a DRAM-contiguous
   buffer; the host undoes the tiling permutation for free.

Layouts inside a core (partition dim first; per 512-token half):
  xT      [128(d%128), 8(d//128), 1024(t)]   moving operand of mm1/router
  w1T/w3T [128(d%128), 8(d//128), 512(h)]    stationary tiles [d,h] for mm1
  h/u     PSUM [128(h%128), 512(t)]          per h-tile, accum over d-tiles
  gu      [128(h%128), 4(h//128), 512(t)]    stationary tiles [h,t] for mm2
  w2T     [128(h%128), 4(h//128), 1024(d)]   moving operand of mm2
  y       PSUM [128(t%128), 512(d-chunk)]    accum over h-tiles
  out_acc [128(t%128), 2(half), 4(tt), 1024(d)]  sum_e cw_e * (y_e + b2_e)
"""

import numpy as np

import concourse.bacc as bacc
import concourse.mybir as mybir
import concourse.tile as tile

D, H, E, T = 1024, 512, 8, 4096
NCORES = 4                  # of the 8 visible; see module docstring
TLOC = T // NCORES          # 1024 tokens per core
NHALF = 2                   # 512-token passes per core
THALF = TLOC // NHALF       # 512
DT = D // 128               # 8 d-tiles
HT = H // 128               # 4 h-tiles
TT = THALF // 128           # 4 t-tiles per half
DC = D // 512               # 2 d-chunks for mm2 moving operand
N_WARM = 5                  # discarded matmuls to ramp the PE clock
F32 = mybir.dt.float32
F32R = mybir.dt.float32r
BF16 = mybir.dt.bfloat16
AX = mybir.AluOpType

# column offsets inside the packed f32 tensor xs [128, XS_COLS]:
# the per-core token slice x^T, then the small operands rwt | b1 | b3 | b2
XS_X = 0                     # [128, DT*TLOC]  x in [p, a, t] order
SM_RWT = XS_X + DT * TLOC    # [128, DT*E]     router_w in [p, a, e] order
SM_B1 = SM_RWT + DT * E      # [128, E*HT]     b1 in [p, e, ht] order
SM_B3 = SM_B1 + E * HT       # [128, E*HT]     b3 in [p, e, ht] order
SM_B2 = SM_B3 + E * HT       # rows 0:E hold b2 [E, D]
XS_COLS = SM_B2 + D


def _bc(ap, n):
    """Append a step-0 (broadcast) innermost free dim of size n."""
    return ap.broadcast_to([*ap.shape, n])


def build_nc():
    nc = bacc.Bacc("TRN2", target_bir_lowering=False, debug=False,
                   num_devices=NCORES)

    xs = nc.dram_tensor("xs", [128, XS_COLS], F32, kind="ExternalInput")
    wb = nc.dram_tensor("wb", [E, 128, 3, DT * H], BF16, kind="ExternalInput")
    out = nc.dram_tensor("out", [NHALF, TT, DC, 128, 512], F32,
                         kind="ExternalOutput")

    with tile.TileContext(nc) as tc:
        with (
            tc.tile_pool(name="singles", bufs=1) as singles,
            tc.tile_pool(name="wpool", bufs=2) as wpool,
            tc.tile_pool(name="gpool", bufs=2) as gpool,
            tc.tile_pool(name="pmm", bufs=6, space="PSUM") as pmm,
            tc.tile_pool(name="psmall", bufs=2, space="PSUM") as psmall,
        ):
            # ---- one-time loads (order = DMA queue order) ------------------
            sm_ap = xs.ap()
            rwt_sb = singles.tile([128, DT, E], F32)
            nc.sync.dma_start(
                out=rwt_sb,
                in_=sm_ap[:, SM_RWT:SM_B1].rearrange("p (a e) -> p a e", a=DT))
            # x lands once as fp32 (router needs true fp32); the bf16 FFN
            # copy is made on-chip by the otherwise-idle DVE
            xtf_sb = singles.tile([128, DT, TLOC], F32)
            xtf_r = sm_ap[:, XS_X:SM_RWT].rearrange("p (a t) -> p a t", a=DT)
            for dt in range(DT):
                nc.sync.dma_start(out=xtf_sb[:, dt, :], in_=xtf_r[:, dt, :])
            xt_sb = singles.tile([128, DT, TLOC], BF16)
            for dt in range(DT):
                nc.vector.tensor_copy(xt_sb[:, dt, :], xtf_sb[:, dt, :])
            b2f_sb = singles.tile([E, D], F32)
            nc.sync.dma_start(out=b2f_sb, in_=sm_ap[0:E, SM_B2:XS_COLS])
            b2_sb = singles.tile([E, D], F32R)
            nc.vector.tensor_copy(b2_sb, b2f_sb)
            b1_sb = singles.tile([128, E, HT], F32)
            nc.sync.dma_start(
                out=b1_sb,
                in_=sm_ap[:, SM_B1:SM_B3].rearrange("p (e h) -> p e h", e=E))
            dume = singles.tile([1, 1], F32)
            nc.scalar.activation(dume, rwt_sb[0:1, 0, 0:1],
                                 mybir.ActivationFunctionType.Exp)

            # ---- PE warm-up: discarded bf16 matmuls ------------------------
            p_warm = psmall.tile([128, THALF], F32, tag="small")
            for _ in range(N_WARM):
                nc.tensor.matmul(p_warm, xt_sb[:, 0, 0:128],
                                 xt_sb[:, 0, 0:THALF],
                                 start=True, stop=True)

            # ---- router (per half): logitsT[e, t] = router_w @ x.T ---------
            # full fp32 so top-2 selection matches the fp32 reference.
            # scores32 doubles as the dense combine-weight tile (cols 8+
            # stay 0); token t = 128*tt + 32*q + i of a half lives at
            # [i, half, 4*tt+q, e].
            lgT32 = singles.tile([32, NHALF, 16, 32], F32)
            scores32 = singles.tile([32, NHALF, 16, 32], F32)
            nc.vector.memset(scores32, 0.0)
            ssum = singles.tile([32, NHALF, 16], F32)
            rsum = singles.tile([32, NHALF, 16], F32)
            m1 = singles.tile([32, NHALF, 16], F32)
            m2 = singles.tile([32, NHALF, 16], F32)
            tmp32 = singles.tile([32, NHALF, 16, E], F32)
            cwTp = singles.tile([32, NHALF, 16, 32], F32)
            cwT = singles.tile([E, NHALF, 16, 32], F32R)
            cw128 = singles.tile([128, NHALF, TT, E], F32)

            def emit_router_half(hf):
                ts_ = slice(hf * THALF, (hf + 1) * THALF)
                p_lg = psmall.tile([32, THALF], F32, tag="small")
                nc.vector.memset(p_lg, 0.0)
                for dt in range(DT):
                    nc.tensor.matmul(p_lg[0:E, :], rwt_sb[:, dt, :],
                                     xtf_sb[:, dt, ts_],
                                     start=(dt == 0), stop=(dt == DT - 1))
                # transpose logitsT straight out of PSUM on the DVE (32x32
                # block transpose) - no PE op in the router->cw chain
                nc.vector.transpose(
                    lgT32[:, hf].rearrange("p a e -> p (a e)"), p_lg)

                # softmax over e (no max-subtraction: logits ~ N(0,1))
                sl = lgT32[:, hf, :, 0:E]
                sc = scores32[:, hf, :, 0:E]
                nc.scalar.activation(sc, sl,
                                     mybir.ActivationFunctionType.Exp)
                nc.vector.reduce_sum(ssum[:, hf], sc, axis=mybir.AxisListType.X)
                nc.vector.reciprocal(rsum[:, hf], ssum[:, hf])
                nc.vector.tensor_tensor(sc, sc, _bc(rsum[:, hf], E), op=AX.mult)

                # top-2: cw = score * (score >= second_max)
                nc.vector.reduce_max(m1[:, hf], sc, axis=mybir.AxisListType.X)
                nc.vector.tensor_tensor(tmp32[:, hf], sc, _bc(m1[:, hf], E),
                                        op=AX.is_equal)
                nc.vector.scalar_tensor_tensor(tmp32[:, hf], tmp32[:, hf],
                                               -1e30, sc,
                                               op0=AX.mult, op1=AX.add)
                nc.vector.reduce_max(m2[:, hf], tmp32[:, hf],
                                     axis=mybir.AxisListType.X)
                nc.vector.tensor_tensor(tmp32[:, hf], sc, _bc(m2[:, hf], E),
                                        op=AX.is_ge)
                nc.vector.tensor_tensor(sc, sc, tmp32[:, hf], op=AX.mult)

                # cwT[e, t] via a second DVE block transpose (rows 8+ junk)
                nc.vector.transpose(
                    cwTp[:, hf].rearrange("p a e -> p (a e)"),
                    scores32[:, hf].rearrange("p a e -> p (a e)"))
                nc.vector.tensor_copy(cwT[:, hf], cwTp[0:E, hf, :, :])

                # cw in [t%128, tt, e] layout for the y-combine scalars:
                # tiny partition-shift DMAs on the gpsimd queue (the sync
                # queue carries the big weight streams and must not
                # head-of-line block on cw)
                cw_v = scores32[:, hf].rearrange("p (t q) e -> p t q e", q=4)
                for q in range(4):
                    nc.gpsimd.dma_start(
                        out=cw128[32 * q:32 * (q + 1), hf, :, :],
                        in_=cw_v[:, :, q, 0:E])

            for hf in range(NHALF):
                emit_router_half(hf)

            def emit_expert_hu(e, hf, w1_sb, w3_sb):
                ts_ = slice(hf * THALF, (hf + 1) * THALF)
                g_sb = gpool.tile([128, HT, THALF], F32, tag="g")
                hb_sb = gpool.tile([128, HT, THALF], F32, tag="hb")
                gu_sb = gpool.tile([128, HT, THALF], BF16, tag="gu")
                for ht in range(HT):
                    hs = slice(ht * 128, (ht + 1) * 128)
                    p_h = pmm.tile([128, THALF], F32, tag="mm")
                    for dt in range(DT):
                        nc.tensor.matmul(p_h, w1_sb[:, dt, hs],
                                         xt_sb[:, dt, ts_],
                                         start=(dt == 0), stop=(dt == DT - 1))
                    # silu(h+b1)*(u+b3) = (h+b1)*sigmoid(h+b1)*(u+b3)
                    nc.scalar.activation(g_sb[:, ht, :], p_h,
                                         mybir.ActivationFunctionType.Sigmoid,
                                         bias=b1_sb[:, e, ht:ht + 1], scale=1.0)
                    nc.vector.tensor_scalar_add(hb_sb[:, ht, :], p_h,
                                                b1_sb[:, e, ht:ht + 1])
                for ht in range(HT):
                    hs = slice(ht * 128, (ht + 1) * 128)
                    p_u = pmm.tile([128, THALF], F32, tag="mm")
                    for dt in range(DT):
                        nc.tensor.matmul(p_u, w3_sb[:, dt, hs],
                                         xt_sb[:, dt, ts_],
                                         start=(dt == 0), stop=(dt == DT - 1))
                    nc.vector.scalar_tensor_tensor(gu_sb[:, ht, :], p_u,
                                                   b3_sb[:, e, ht:ht + 1],
                                                   g_sb[:, ht, :],
                                                   op0=AX.add, op1=AX.mult)
                    nc.vector.tensor_mul(gu_sb[:, ht, :], gu_sb[:, ht, :],
                                         hb_sb[:, ht, :])
                return gu_sb

            def emit_expert_y(e, hf, gu_sb, w2_sb):
                # y[t, d] = gu.T @ w2T ; out_acc += cw_e * y
                for tt in range(TT):
                    ts_ = slice(tt * 128, (tt + 1) * 128)
                    for dc in range(DC):
                        ds_ = slice(dc * 512, (dc + 1) * 512)
                        p_y = pmm.tile([128, 512], F32, tag="mm")
                        for ht in range(HT):
                            nc.tensor.matmul(p_y, gu_sb[:, ht, ts_],
                                             w2_sb[:, ht, ds_],
                                             start=(ht == 0),
                                             stop=(ht == HT - 1))
                        nc.vector.scalar_tensor_tensor(
                            out_acc[:, hf, tt, ds_], p_y,
                            cw128[:, hf, tt, e:e + 1],
                            out_acc[:, hf, tt, ds_], op0=AX.mult, op1=AX.add)

            def emit_expert_dmas(e):
                wb_e = wb.ap()[e]
                w1_sb = wpool.tile([128, DT, H], BF16, tag="w1")
                nc.sync.dma_start(
                    out=w1_sb,
                    in_=wb_e[:, 0, :].rearrange("p (a h) -> p a h", a=DT))
                if e == 0:
                    nc.sync.dma_start(
                        out=b3_sb,
                        in_=xs.ap()[:, SM_B3:SM_B2].rearrange(
                            "p (e h) -> p e h", e=E))
                w3_sb = wpool.tile([128, DT, H], BF16, tag="w3")
                nc.sync.dma_start(
                    out=w3_sb,
                    in_=wb_e[:, 1, :].rearrange("p (a h) -> p a h", a=DT))
                w2_sb = wpool.tile([128, HT, D], BF16, tag="w2")
                nc.sync.dma_start(
                    out=w2_sb,
                    in_=wb_e[:, 2, :].rearrange("p (a d) -> p a d", a=HT))
                return w1_sb, w3_sb, w2_sb

            for _ in range(12):
                nc.tensor.matmul(p_warm, xt_sb[:, 0, 0:128],
                                 xt_sb[:, 0, 0:THALF],
                                 start=True, stop=True)

            # out_acc = cw @ b2 (the bias part of the combine)
            b3_sb = singles.tile([128, E, HT], F32)
            out_acc = singles.tile([128, NHALF, TT, D], F32)
            for hf in range(NHALF):
                for tt in range(TT):
                    for dc in range(DC):
                        p_b = pmm.tile([128, 512], F32, tag="mm")
                        nc.tensor.matmul(p_b,
                                         cwT[:, hf, 4 * tt:4 * (tt + 1), :],
                                         b2_sb[:, dc * 512:(dc + 1) * 512])
                        nc.vector.tensor_copy(
                            out_acc[:, hf, tt, dc * 512:(dc + 1) * 512], p_b)

            for e in range(E):
                w1_sb, w3_sb, w2_sb = emit_expert_dmas(e)
                for hf in range(NHALF):
                    gu_sb = emit_expert_hu(e, hf, w1_sb, w3_sb)
                    emit_expert_y(e, hf, gu_sb, w2_sb)

            # ---- store (chunked + DRAM-contiguous; host re-lays-out) -------
            out_r = out.ap().rearrange("f a b p d -> p f a b d")
            for hf in range(NHALF):
                for tt in range(TT):
                    for dc in range(DC):
                        nc.sync.dma_start(
                            out=out_r[:, hf, tt, dc, :],
                            in_=out_acc[:, hf, tt, dc * 512:(dc + 1) * 512])

    nc.compile()
    return nc


_NC_CACHE = None


def _get_nc():
    global _NC_CACHE
    if _NC_CACHE is None:
        _NC_CACHE = build_nc()
    return _NC_CACHE


def make_in_maps(x, router_w, w1, b1, w3, b3, w2, b2):
    import ml_dtypes

    bf16 = ml_dtypes.bfloat16
    f32 = np.float32

    # packed f32 tensor: x^T (filled per core below) | rwt | b1 | b3 | b2
    xs = np.zeros((128, XS_COLS), f32)
    xs[:, SM_RWT:SM_B1] = (router_w.T.astype(f32)
                           .reshape(DT, 128, E).transpose(1, 0, 2)
                           .reshape(128, DT * E))
    xs[:, SM_B1:SM_B3] = (b1.astype(f32)
                          .reshape(E, HT, 128).transpose(2, 0, 1)
                          .reshape(128, E * HT))
    xs[:, SM_B3:SM_B2] = (b3.astype(f32)
                          .reshape(E, HT, 128).transpose(2, 0, 1)
                          .reshape(128, E * HT))
    xs[0:E, SM_B2:XS_COLS] = b2.astype(f32)

    # packed bf16 weights, one [128, 3, DT*H] block per expert, already in
    # the SBUF layout ([d%128, d//128, h] for w1/w3, [h%128, h//128, d] for w2)
    def perm(w, kt):  # [E, out, in] -> [E, 128, kt * in]
        return (w.transpose(0, 2, 1).reshape(E, kt, 128, -1)
                .transpose(0, 2, 1, 3).reshape(E, 128, kt * w.shape[1]))

    wbn = np.empty((E, 128, 3, DT * H), bf16)
    wbn[:, :, 0, :] = perm(w1.astype(f32), DT)
    wbn[:, :, 1, :] = perm(w3.astype(f32), DT)
    wbn[:, :, 2, :] = perm(w2.astype(f32), HT)

    xt_full = np.ascontiguousarray(x.astype(f32).reshape(T, D))
    in_maps = []
    for c in range(NCORES):
        xc = xt_full[c * TLOC:(c + 1) * TLOC]
        xsc = xs.copy()
        xsc[:, XS_X:SM_RWT] = (xc.T.reshape(DT, 128, TLOC)
                               .transpose(1, 0, 2).reshape(128, DT * TLOC))
        in_maps.append({"xs": xsc, "wb": wbn})
    return in_maps


def core0_slice(out_arr):
    """Undo the on-device [NHALF, TT, DC, 128, 512] tiling for one core."""
    return out_arr.transpose(0, 1, 3, 2, 4).reshape(TLOC, D)


def kernel(x, router_w, w1, b1, w3, b3, w2, b2):
    from concourse.bass_utils import run_bass_kernel_spmd

    nc = _get_nc()
    in_maps = make_in_maps(np.asarray(x, dtype=np.float32),
                           np.asarray(router_w, dtype=np.float32),
                           np.asarray(w1, dtype=np.float32),
                           np.asarray(b1, dtype=np.float32),
                           np.asarray(w3, dtype=np.float32),
                           np.asarray(b3, dtype=np.float32),
                           np.asarray(w2, dtype=np.float32),
                           np.asarray(b2, dtype=np.float32))
    res = run_bass_kernel_spmd(nc, in_maps, core_ids=list(range(NCORES)))
    outs = [core0_slice(res.results[c]["out"]) for c in range(NCORES)]
    return np.concatenate(outs, axis=0).reshape(4, 1024, D)
